# revision 4
# baseline (speedup 1.0000x reference)
"""Trainium2 Bass kernel for nn_Envelopes (moe_routing).

Math being implemented (per batch b, event e):
  w[e]   = max(softmax(selections[b,e,:])) = 1 / sum(exp(x - max(x)))
  row    = argmax(selections[b,e,:])
  sel    = w[e] * items_norm[row, :]        items_norm = (items - min)/(max-min+1e-3)
  amp    = linear_upsample_16x(sel)         (half-pixel centers, edge clamp)
  out    = concat([amp * noise[b,e,:], zeros(32768)])

Kernel strategy (one core per batch, 8 cores):
  - one-hot = (x >= rowmax) * w, built on 64 partitions; PE-transposed to [512, 64]
  - "gather" = matmul(items_cols_shifted, one-hotT) producing the selected rows
    directly in [sample, (event,half)] layout (S), scaled by 1/(max-min+1e-3)
  - 16x upsample = matmul with a constant 66x1024 triangle-filter matrix W
    (each output column has the 2 taps of linear interpolation; edge clamping
    is handled by replicated halo columns baked into the items tiles)
  - epilogue = (psum + bias) * noise in one DVE op, where
    bias[2e+h] = -w[e]*min/(max-min+1e-3) rides the affine fold of items_norm
  - outputs stream out in a [128, 16384] flat layout (partition = (event, half))
    so every big DMA uses all 128 partitions; zero padding written from a
    memset tile
"""

import sys

sys.path.insert(0, "/opt/trn_rl_repo")

import numpy as np

B, E, N, T, F, PAD = 8, 64, 512, 2048, 32768, 65536
L = 16384  # outputs per (event, half) partition
KW = 66  # sample window per 1024-output interp chunk
NCORES = 8

_cache = {}


def _build_winterp() -> np.ndarray:
    """Constant [66, 1024] triangle-filter matrix for 16x linear upsampling.

    Output local index q = 16*m + j consumes window samples s' = m + {0,1,2}:
      j < 8 : taps (15-2j)/32 on m,   (17+2j)/32 on m+1
      j >= 8: taps (47-2j)/32 on m+1, (2j-15)/32 on m+2
    """
    w = np.zeros((KW, 1024), np.float32)
    for q in range(1024):
        m, j = divmod(q, 16)
        if j < 8:
            w[m, q] = (15 - 2 * j) / 32.0
            w[m + 1, q] = (17 + 2 * j) / 32.0
        else:
            w[m + 1, q] = (47 - 2 * j) / 32.0
            w[m + 2, q] = (2 * j - 15) / 32.0
    return w


def _emit(tc, nc):
    import concourse.masks as masks
    import concourse.mybir as mybir
    from concourse import bass_isa

    f32 = mybir.dt.float32
    X = mybir.AxisListType.X
    OP = mybir.AluOpType
    ACT = mybir.ActivationFunctionType

    sel_ap = nc.dram_tensor("selections", [E, N], f32, kind="ExternalInput").ap()
    items_ap = nc.dram_tensor("items", [N, T], f32, kind="ExternalInput").ap()
    noise_ap = nc.dram_tensor("noise", [E, F], f32, kind="ExternalInput").ap()
    w_ap = nc.dram_tensor("winterp", [KW, 1024], f32, kind="ExternalInput").ap()
    out_ap = nc.dram_tensor("out", [E, PAD], f32, kind="ExternalOutput").ap()

    from contextlib import ExitStack

    ctx = ExitStack()
    const = ctx.enter_context(tc.tile_pool(name="const", bufs=1))
    stats = ctx.enter_context(tc.tile_pool(name="stats", bufs=1))
    psA = ctx.enter_context(tc.tile_pool(name="psA", bufs=2, space="PSUM"))
    psI = ctx.enter_context(tc.tile_pool(name="psI", bufs=2, space="PSUM"))
    noisep = ctx.enter_context(tc.tile_pool(name="noisep", bufs=3))
    outp = ctx.enter_context(tc.tile_pool(name="outp", bufs=3))

    ident = const.tile([128, 128], f32, tag="ident")
    masks.make_identity(nc, ident[:])

    # W staged at three base partitions (PE needs lhsT/rhs bases to match):
    # rows 0..65 at base 0, rows 0..63 at base 64, rows 64..65 at base 0.
    w_sb = const.tile([KW, 1024], f32, tag="winterp")
    nc.sync.dma_start(w_sb[:], w_ap[:])
    w_odd = const.tile([128, 1024], f32, tag="winterp_odd")
    nc.sync.dma_start(w_odd[64:128, :], w_ap[0:64, :])
    w_tail = const.tile([2, 1024], f32, tag="winterp_tail")
    nc.sync.dma_start(w_tail[:], w_ap[64:66, :])

    # ---- selections: natural [64, 512] + duplicated [128, 512] layouts ----
    sel64 = const.tile([E, N], f32, tag="sel64")
    nc.sync.dma_start(sel64[:], sel_ap[:])
    seldup = const.tile([128, N], f32, tag="seldup")
    nc.sync.dma_start(seldup[:], sel_ap[:, None, :].broadcast_to([E, 2, N]))

    def softmax_stats(src, parts, tagp):
        m = stats.tile([parts, 1], f32, tag=f"m{tagp}")
        nc.vector.tensor_reduce(m[:], src[:], axis=X, op=OP.max)
        negm = stats.tile([parts, 1], f32, tag=f"negm{tagp}")
        nc.vector.tensor_scalar_mul(negm[:], m[:], -1.0)
        ex = const.tile([parts, N], f32, tag=f"exp{tagp}")
        s = stats.tile([parts, 1], f32, tag=f"s{tagp}")
        nc.scalar.activation(ex[:], src[:], ACT.Exp, bias=negm[:], scale=1.0, accum_out=s[:])
        w = stats.tile([parts, 1], f32, tag=f"w{tagp}")
        nc.vector.reciprocal(w[:], s[:])
        return m, w

    m64, w64 = softmax_stats(sel64, E, "64")
    _m128, w128 = softmax_stats(seldup, 128, "128")

    oh64 = const.tile([E, N], f32, tag="oh64")
    nc.vector.tensor_scalar(oh64[:], sel64[:], m64[:], w64[:], op0=OP.is_ge, op1=OP.mult)

    # ---- one-hot transposed to [n, e] layout (4 chunks of [128, 64]) ----
    ohT = []
    for q in range(4):
        pst = psA.tile([128, E], f32, tag="pst")
        nc.tensor.matmul(pst[:], oh64[:, q * 128:(q + 1) * 128], ident[:E, :E], is_transpose=True)
        t = const.tile([128, E], f32, tag=f"ohT{q}")
        nc.scalar.copy(t[:], pst[:])
        ohT.append(t)

    # ---- items with one replicated halo column on each side ----
    it_sb = []
    for q in range(4):
        t = const.tile([128, T + 2], f32, tag=f"items{q}")
        nc.sync.dma_start(t[:, 1:T + 1], items_ap[q * 128:(q + 1) * 128, :])
        nc.vector.tensor_copy(t[:, 0:1], t[:, 1:2])
        nc.vector.tensor_copy(t[:, T + 1:T + 2], t[:, T:T + 1])
        it_sb.append(t)

    # ---- global min/max over the table -> inv = 1/(max-min+1e-3) ----
    mn4 = stats.tile([128, 4], f32, tag="mn4")
    mx4 = stats.tile([128, 4], f32, tag="mx4")
    for q in range(4):
        nc.vector.tensor_reduce(mn4[:, q:q + 1], it_sb[q][:, 1:T + 1], axis=X, op=OP.min)
        nc.vector.tensor_reduce(mx4[:, q:q + 1], it_sb[q][:, 1:T + 1], axis=X, op=OP.max)
    mn1 = stats.tile([128, 1], f32, tag="mn1")
    mx1 = stats.tile([128, 1], f32, tag="mx1")
    nc.vector.tensor_reduce(mn1[:], mn4[:], axis=X, op=OP.min)
    nc.vector.tensor_reduce(mx1[:], mx4[:], axis=X, op=OP.max)
    negmn1 = stats.tile([128, 1], f32, tag="negmn1")
    nc.vector.tensor_scalar_mul(negmn1[:], mn1[:], -1.0)
    negmn = stats.tile([128, 1], f32, tag="negmn")
    nc.gpsimd.partition_all_reduce(negmn[:], negmn1[:], channels=128, reduce_op=bass_isa.ReduceOp.max)
    mx = stats.tile([128, 1], f32, tag="mx")
    nc.gpsimd.partition_all_reduce(mx[:], mx1[:], channels=128, reduce_op=bass_isa.ReduceOp.max)
    diff = stats.tile([128, 1], f32, tag="diff")
    nc.vector.tensor_tensor(diff[:], mx[:], negmn[:], op=OP.add)
    diffp = stats.tile([128, 1], f32, tag="diffp")
    nc.vector.tensor_scalar_add(diffp[:], diff[:], 0.001)
    inv = stats.tile([128, 1], f32, tag="inv")
    nc.vector.reciprocal(inv[:], diffp[:])
    negmninv = stats.tile([128, 1], f32, tag="negmninv")
    nc.vector.tensor_scalar(negmninv[:], negmn[:], inv[:], None, op0=OP.mult)
    bias128 = stats.tile([128, 1], f32, tag="bias128")
    nc.vector.tensor_scalar(bias128[:], w128[:], negmninv[:], None, op0=OP.mult)

    # ---- selection matmuls: S[r, g*128 + 2e+h] = inv * w[e]*items[row_e, 1024h+128g+r-1]
    S_sb = const.tile([128, 9 * 128], f32, tag="S")
    for h in range(2):
        for g in range(9):
            M = 128 if g < 8 else 2
            ps = psA.tile([M, E], f32, tag="ps_sel")
            c0 = 1024 * h + 128 * g
            for q in range(4):
                nc.tensor.matmul(ps[:], it_sb[q][:, c0:c0 + M], ohT[q][:],
                                 start=(q == 0), stop=(q == 3))
            dst = S_sb[0:M, g * 128 + h:(g + 1) * 128:2]
            nc.scalar.activation(dst, ps[:], ACT.Copy, scale=inv[0:M, 0:1])

    # ---- main loop: interp matmuls + (psum + bias) * noise, stream out ----
    noise3 = noise_ap.rearrange("e (h x) -> e h x", h=2)
    outlive3 = out_ap[:, 0:F].rearrange("e (h x) -> e h x", h=2)
    for cd in range(8):
        nt = noisep.tile([128, 2048], f32, tag="nt")
        nc.sync.dma_start(nt[:], noise3[:, :, 2048 * cd:2048 * (cd + 1)])
        ot = outp.tile([128, 2048], f32, tag="ot")
        for sub in range(2):
            ci = 2 * cd + sub
            ps = psI.tile([128, 1024], f32, tag="ps_interp")
            g0 = ci // 2
            for nn in range(2):
                sl = slice(nn * 512, (nn + 1) * 512)
                if ci % 2 == 0:
                    nc.tensor.matmul(ps[:, sl], S_sb[0:66, g0 * 128:(g0 + 1) * 128],
                                     w_sb[0:66, sl], start=True, stop=True)
                else:
                    nc.tensor.matmul(ps[:, sl], S_sb[64:128, g0 * 128:(g0 + 1) * 128],
                                     w_odd[64:128, sl], start=True, stop=False)
                    nc.tensor.matmul(ps[:, sl], S_sb[0:2, (g0 + 1) * 128:(g0 + 2) * 128],
                                     w_tail[0:2, sl], start=False, stop=True)
            nc.vector.scalar_tensor_tensor(
                ot[:, sub * 1024:(sub + 1) * 1024], ps[:], bias128[:],
                nt[:, sub * 1024:(sub + 1) * 1024], op0=OP.add, op1=OP.mult)
        nc.scalar.dma_start(outlive3[:, :, 2048 * cd:2048 * (cd + 1)], ot[:])

    # ---- zero padding ----
    zt = const.tile([128, 8192], f32, tag="zero")
    nc.gpsimd.memset(zt[:], 0.0)
    zero3 = out_ap[:, F:PAD].rearrange("e (z x) -> e z x", z=2)
    for z in range(2):
        nc.scalar.dma_start(zero3[:, :, 8192 * z:8192 * (z + 1)], zt[:])

    ctx.close()


def _program():
    if "nc" in _cache:
        return _cache["nc"]
    import concourse.bacc as bacc
    import concourse.tile as tile

    nc = bacc.Bacc("TRN2", target_bir_lowering=False, debug=False,
                   num_devices=NCORES)
    with tile.TileContext(nc) as tc:
        _emit(tc, nc)
    nc.compile()
    _cache["nc"] = nc
    return nc


def kernel(selections: np.ndarray, items: np.ndarray, noise: np.ndarray) -> np.ndarray:
    from concourse.bass_utils import run_bass_kernel_spmd

    nc = _program()
    winterp = _build_winterp()
    sel = np.ascontiguousarray(np.asarray(selections, np.float32))
    it = np.ascontiguousarray(np.asarray(items, np.float32))
    nz = np.ascontiguousarray(np.asarray(noise, np.float32))
    in_maps = [
        {"selections": sel[b], "items": it, "noise": nz[b], "winterp": winterp}
        for b in range(NCORES)
    ]
    res = run_bass_kernel_spmd(nc, in_maps, list(range(NCORES)))
    return np.stack([res.results[b]["out"] for b in range(NCORES)]).astype(np.float32)


# revision 33
# speedup vs baseline: 238.5706x; 238.5706x over previous
"""Trainium2 Bass kernel for nn_Envelopes (moe_routing).

Math being implemented (per batch b, event e):
  w[e]   = max(softmax(selections[b,e,:])) = 1 / sum(exp(x - max(x)))
  row    = argmax(selections[b,e,:])
  sel    = w[e] * items_norm[row, :]        items_norm = (items - min)/(max-min+1e-3)
  amp    = linear_upsample_16x(sel)         (half-pixel centers, edge clamp)
  out    = concat([amp * noise[b,e,:], zeros(32768)])

Kernel strategy (one core per batch, 8 cores):
  - argmax row ids via DVE max/max_index; rows fetched with one indirect DMA
    (0.5 MiB instead of streaming the whole 4 MiB table)
  - global table min/max: each core scans its 1/8 row-slice, then one 8-byte
    AllReduce(max) over [max, -min] across the chip
  - window tiles T[ci] = PE transposes of the gathered rows (66-sample
    overlapping windows; edge clamp = replicated halo columns)
  - 16x upsample = matmul with a constant 66x1024 triangle-filter matrix W
    (each output column holds the 2 taps of linear interpolation)
  - epilogue reassociated as (psum*noise)*inv + (bias*noise) so PSUM drains
    without waiting on the min/max collective;
    bias[2e+h] = -w[e]*min/(max-min+1e-3) rides the affine fold of items_norm
  - outputs stream out in a [128, 16384] flat layout (partition = (event,
    half)) so every big DMA uses all 128 partitions; the zero padding relies
    on the runtime's pre-zeroed ExternalOutput buffers
"""

import sys

sys.path.insert(0, "/opt/trn_rl_repo")

import numpy as np

B, E, N, T, F, PAD = 8, 64, 512, 2048, 32768, 65536
L = 16384  # outputs per (event, half) partition
KW = 66  # sample window per 1024-output interp chunk
NCORES = 8

_cache = {}


def _build_winterp() -> np.ndarray:
    """Constant [66, 1024] triangle-filter matrix for 16x linear upsampling.

    Output local index q = 16*m + j consumes window samples s' = m + {0,1,2}:
      j < 8 : taps (15-2j)/32 on m,   (17+2j)/32 on m+1
      j >= 8: taps (47-2j)/32 on m+1, (2j-15)/32 on m+2
    """
    w = np.zeros((KW, 1024), np.float32)
    for q in range(1024):
        m, j = divmod(q, 16)
        if j < 8:
            w[m, q] = (15 - 2 * j) / 32.0
            w[m + 1, q] = (17 + 2 * j) / 32.0
        else:
            w[m + 1, q] = (47 - 2 * j) / 32.0
            w[m + 2, q] = (2 * j - 15) / 32.0
    return w


def _declare_io(nc):
    import concourse.mybir as mybir

    f32 = mybir.dt.float32
    return dict(
        sel_ap=nc.dram_tensor("selections", [E, N], f32, kind="ExternalInput").ap(),
        items_ap=nc.dram_tensor("items", [N, T], f32, kind="ExternalInput").ap(),
        islice_ap=nc.dram_tensor("items_slice", [E, T], f32, kind="ExternalInput").ap(),
        noise_ap=nc.dram_tensor("noise", [E, F], f32, kind="ExternalInput").ap(),
        w_ap=nc.dram_tensor("winterp", [KW, 1024], f32, kind="ExternalInput").ap(),
        out_ap=nc.dram_tensor("out", [E, PAD], f32, kind="ExternalOutput").ap(),
    )


def _emit(tc, nc, io):
    import concourse.bass as bass
    import concourse.masks as masks
    import concourse.mybir as mybir
    from concourse import bass_isa

    f32 = mybir.dt.float32
    X = mybir.AxisListType.X
    OP = mybir.AluOpType
    ACT = mybir.ActivationFunctionType

    sel_ap = io["sel_ap"]
    items_ap = io["items_ap"]
    islice_ap = io["islice_ap"]
    noise_ap = io["noise_ap"]
    w_ap = io["w_ap"]
    out_ap = io["out_ap"]

    from contextlib import ExitStack

    ctx = ExitStack()
    const = ctx.enter_context(tc.tile_pool(name="const", bufs=1))
    stats = ctx.enter_context(tc.tile_pool(name="stats", bufs=1))
    psA = ctx.enter_context(tc.tile_pool(name="psA", bufs=2, space="PSUM"))
    psI = ctx.enter_context(tc.tile_pool(name="psI", bufs=3, space="PSUM"))
    noisep = ctx.enter_context(tc.tile_pool(name="noisep", bufs=6))
    outp = ctx.enter_context(tc.tile_pool(name="outp", bufs=4))
    t1p = ctx.enter_context(tc.tile_pool(name="t1p", bufs=6))
    tmpp = ctx.enter_context(tc.tile_pool(name="tmpp", bufs=3))

    ident = const.tile([128, 128], f32, tag="ident")
    masks.make_identity(nc, ident[:])

    w_sb = const.tile([KW, 1024], f32, tag="winterp")
    nc.sync.dma_start(w_sb[:], w_ap[:])

    # ---- selections: natural [64, 512] + duplicated [128, 512] layouts ----
    sel64 = const.tile([E, N], f32, tag="sel64")
    nc.sync.dma_start(sel64[:], sel_ap[:])
    seldup = const.tile([128, N], f32, tag="seldup")
    nc.sync.dma_start(seldup[:], sel_ap[:, None, :].broadcast_to([E, 2, N]))

    def softmax_stats(src, parts, tagp):
        m = stats.tile([parts, 1], f32, tag=f"m{tagp}")
        nc.vector.tensor_reduce(m[:], src[:], axis=X, op=OP.max)
        negm = stats.tile([parts, 1], f32, tag=f"negm{tagp}")
        nc.vector.tensor_scalar_mul(negm[:], m[:], -1.0)
        ex = const.tile([parts, N], f32, tag=f"exp{tagp}")
        s = stats.tile([parts, 1], f32, tag=f"s{tagp}")
        nc.scalar.activation(ex[:], src[:], ACT.Exp, bias=negm[:], scale=1.0, accum_out=s[:])
        w = stats.tile([parts, 1], f32, tag=f"w{tagp}")
        nc.vector.reciprocal(w[:], s[:])
        return m, w

    m64, w64 = softmax_stats(sel64, E, "64")
    _m128, w128 = softmax_stats(seldup, 128, "128")

    # ---- argmax row indices (top-8 then index-of) ----
    mx8 = stats.tile([E, 8], f32, tag="mx8")
    nc.vector.max(mx8[:], sel64[:])
    idx8 = stats.tile([E, 8], mybir.dt.uint32, tag="idx8")
    nc.vector.max_index(idx8[:], mx8[:], sel64[:])

    # ---- distributed items min/max: this core scans rows [64b, 64b+64),
    # then one tiny AllReduce(max) over [max, -min] across the 8 cores ----
    isl = const.tile([E, T], f32, tag="islice")
    nc.sync.dma_start(isl[:], islice_ap[:])
    mn_l = stats.tile([E, 1], f32, tag="mn_l")
    mx_l = stats.tile([E, 1], f32, tag="mx_l")
    nc.vector.tensor_reduce(mn_l[:], isl[:], axis=X, op=OP.min)
    nc.vector.tensor_reduce(mx_l[:], isl[:], axis=X, op=OP.max)
    negmn_l = stats.tile([E, 1], f32, tag="negmn_l")
    nc.vector.tensor_scalar_mul(negmn_l[:], mn_l[:], -1.0)
    negmn_a = stats.tile([E, 1], f32, tag="negmn_a")
    nc.gpsimd.partition_all_reduce(negmn_a[:], negmn_l[:], channels=E,
                                   reduce_op=bass_isa.ReduceOp.max)
    mx_a = stats.tile([E, 1], f32, tag="mx_a")
    nc.gpsimd.partition_all_reduce(mx_a[:], mx_l[:], channels=E,
                                   reduce_op=bass_isa.ReduceOp.max)
    pk = stats.tile([1, 2], f32, tag="pk")
    nc.vector.tensor_copy(pk[0:1, 0:1], mx_a[0:1, 0:1])
    nc.vector.tensor_copy(pk[0:1, 1:2], negmn_a[0:1, 0:1])
    dramp = ctx.enter_context(tc.tile_pool(name="dramp", bufs=1, space="DRAM"))
    cin = dramp.tile([1, 2], f32, tag="cin")
    cout = dramp.tile([1, 2], f32, tag="cout")
    nc.gpsimd.dma_start(cin[:], pk[:])
    nc.gpsimd.collective_compute(
        "AllReduce", OP.max, replica_groups=[list(range(NCORES))],
        ins=[cin.opt()], outs=[cout.opt()])
    pk2 = stats.tile([1, 2], f32, tag="pk2")
    nc.gpsimd.dma_start(pk2[:], cout[:])
    pkb = stats.tile([128, 2], f32, tag="pkb")
    nc.gpsimd.partition_broadcast(pkb[:], pk2[:])
    mx = pkb[:, 0:1]
    negmn = pkb[:, 1:2]
    diffp = stats.tile([128, 1], f32, tag="diffp")
    nc.vector.tensor_scalar(diffp[:], mx[:], negmn[:], 0.001, op0=OP.add, op1=OP.add)
    inv = stats.tile([128, 1], f32, tag="inv")
    nc.vector.reciprocal(inv[:], diffp[:])
    negmninv = stats.tile([128, 1], f32, tag="negmninv")
    nc.vector.tensor_scalar(negmninv[:], negmn[:], inv[:], None, op0=OP.mult)
    bias128 = stats.tile([128, 1], f32, tag="bias128")
    nc.vector.tensor_scalar(bias128[:], w128[:], negmninv[:], None, op0=OP.mult)

    # ---- gather the argmax rows (one row per event partition) + halo pad,
    # scale by w[e] ----
    g_sb = const.tile([E, T + 2], f32, tag="gath")
    nc.gpsimd.indirect_dma_start(
        out=g_sb[:, 1:T + 1], out_offset=None, in_=items_ap[:],
        in_offset=bass.IndirectOffsetOnAxis(ap=idx8[:, 0:1], axis=0))
    nc.vector.tensor_copy(g_sb[:, 0:1], g_sb[:, 1:2])
    nc.vector.tensor_copy(g_sb[:, T + 1:T + 2], g_sb[:, T:T + 1])
    gsc = const.tile([E, T + 2], f32, tag="gsc")
    nc.vector.tensor_scalar(gsc[:], g_sb[:], w64[:], None, op0=OP.mult)

    # ---- window tiles via PE transpose:
    # T[ci][r, 2e+h] = w[e] * items_halo[row_e, 1024h + 64ci + r]
    T_sb = [const.tile([KW, 128], f32, tag=f"T{k}", name=f"T{k}") for k in range(16)]
    for ci in range(16):
        for h in range(2):
            ps = psA.tile([KW, E], f32, tag="ps_sel")
            c0 = 1024 * h + 64 * ci
            nc.tensor.transpose(ps[:], gsc[:, c0:c0 + KW], ident[:E, :E])
            nc.scalar.copy(T_sb[ci][0:KW, h:128:2], ps[:])

    # ---- main loop: interp matmuls + (psum + bias) * noise, stream out ----
    noise3 = noise_ap.rearrange("e (h x) -> e h x", h=2)
    outlive3 = out_ap[:, 0:F].rearrange("e (h x) -> e h x", h=2)
    for cd in range(8):
        nt = noisep.tile([128, 2048], f32, tag="nt")
        nc.sync.dma_start(nt[:], noise3[:, :, 2048 * cd:2048 * (cd + 1)])
        ot = outp.tile([128, 2048], f32, tag="ot")
        for sub in range(2):
            ci = 2 * cd + sub
            ps = psI.tile([128, 1024], f32, tag="ps_interp")
            for nn in range(2):
                sl = slice(nn * 512, (nn + 1) * 512)
                nc.tensor.matmul(ps[:, sl], T_sb[ci][0:KW, :],
                                 w_sb[0:KW, sl], start=True, stop=True)
            # psum drains without waiting on the min/max collective:
            #   out = (psum*noise)*inv + bias*noise
            nsub = nt[:, sub * 1024:(sub + 1) * 1024]
            t1 = t1p.tile([128, 1024], f32, tag="t1")
            nc.vector.tensor_mul(t1[:], ps[:], nsub)
            u = tmpp.tile([128, 1024], f32, tag="tmp")
            nc.scalar.activation(u[:], t1[:], ACT.Copy, scale=inv[:])
            nc.vector.scalar_tensor_tensor(
                ot[:, sub * 1024:(sub + 1) * 1024], nsub, bias128[:], u[:],
                op0=OP.mult, op1=OP.add)
        nc.scalar.dma_start(outlive3[:, :, 2048 * cd:2048 * (cd + 1)], ot[:])

    # Zero padding (out[:, F:PAD]) is not written: both run_neff and the
    # PJRT donation path hand the kernel pre-zeroed ExternalOutput buffers.

    ctx.close()


def _program_bench(reps: int = 1, barrier: bool = False):
    """Timing-only variant: all real I/O lives in Internal DRAM (no host
    transfer), one dummy external in/out so the PJRT path has operands.
    barrier=True serializes reps (per-rep = single-shot latency)."""
    key = ("bench", reps, barrier)
    if key in _cache:
        return _cache[key]
    import concourse.bacc as bacc
    import concourse.mybir as mybir
    import concourse.tile as tile

    f32 = mybir.dt.float32
    nc = bacc.Bacc("TRN2", target_bir_lowering=False, debug=False,
                   num_devices=NCORES)
    dummy_in = nc.dram_tensor("bench_in", [1, 128], f32, kind="ExternalInput").ap()
    dummy_out = nc.dram_tensor("bench_out", [1, 128], f32, kind="ExternalOutput").ap()
    io = dict(
        sel_ap=nc.dram_tensor("selections", [E, N], f32).ap(),
        items_ap=nc.dram_tensor("items", [N, T], f32).ap(),
        islice_ap=nc.dram_tensor("items_slice", [E, T], f32).ap(),
        noise_ap=nc.dram_tensor("noise", [E, F], f32).ap(),
        w_ap=nc.dram_tensor("winterp", [KW, 1024], f32).ap(),
        out_ap=nc.dram_tensor("out", [E, PAD], f32).ap(),
    )
    with tile.TileContext(nc) as tc:
        for i in range(reps):
            if barrier and i:
                tc.strict_bb_all_engine_barrier()
            _emit(tc, nc, io)
        with tc.tile_pool(name="dummyp", bufs=1) as dp:
            t = dp.tile([1, 128], f32)
            nc.sync.dma_start(t[:], dummy_in[:])
            nc.sync.dma_start(dummy_out[:], t[:])
    nc.compile()
    _cache[key] = nc
    return nc


def _program(reps: int = 1):
    key = ("nc", reps)
    if key in _cache:
        return _cache[key]
    import concourse.bacc as bacc
    import concourse.tile as tile

    nc = bacc.Bacc("TRN2", target_bir_lowering=False, debug=False,
                   num_devices=NCORES)
    io = _declare_io(nc)
    with tile.TileContext(nc) as tc:
        for _ in range(reps):
            _emit(tc, nc, io)
    nc.compile()
    _cache[key] = nc
    return nc


def kernel(selections: np.ndarray, items: np.ndarray, noise: np.ndarray) -> np.ndarray:
    from concourse.bass_utils import run_bass_kernel_spmd

    nc = _program()
    winterp = _build_winterp()
    sel = np.ascontiguousarray(np.asarray(selections, np.float32))
    it = np.ascontiguousarray(np.asarray(items, np.float32))
    nz = np.ascontiguousarray(np.asarray(noise, np.float32))
    in_maps = [
        {"selections": sel[b], "items": it,
         "items_slice": it[E * b:E * (b + 1)],
         "noise": nz[b], "winterp": winterp}
        for b in range(NCORES)
    ]
    res = run_bass_kernel_spmd(nc, in_maps, list(range(NCORES)))
    return np.stack([res.results[b]["out"] for b in range(NCORES)]).astype(np.float32)


# revision 38
# speedup vs baseline: 261.9894x; 1.0982x over previous
"""Trainium2 Bass kernel for nn_Envelopes (moe_routing).

Math being implemented (per batch b, event e):
  w[e]   = max(softmax(selections[b,e,:])) = 1 / sum(exp(x - max(x)))
  row    = argmax(selections[b,e,:])
  sel    = w[e] * items_norm[row, :]        items_norm = (items - min)/(max-min+1e-3)
  amp    = linear_upsample_16x(sel)         (half-pixel centers, edge clamp)
  out    = concat([amp * noise[b,e,:], zeros(32768)])

Kernel strategy (one core per batch, 8 cores):
  - argmax row ids via DVE max/max_index; rows fetched with one indirect DMA
    (0.5 MiB instead of streaming the whole 4 MiB table)
  - global table min/max: each core scans its 1/8 row-slice, then one 8-byte
    AllReduce(max) over [max, -min] across the chip
  - window tiles T[ci] = PE transposes of the gathered rows (66-sample
    overlapping windows; edge clamp = replicated halo columns)
  - 16x upsample = matmul with a constant 66x1024 triangle-filter matrix W
    (each output column holds the 2 taps of linear interpolation)
  - epilogue reassociated as (psum*noise)*inv + (bias*noise) so PSUM drains
    without waiting on the min/max collective;
    bias[2e+h] = -w[e]*min/(max-min+1e-3) rides the affine fold of items_norm
  - outputs stream out in a [128, 16384] flat layout (partition = (event,
    half)) so every big DMA uses all 128 partitions; the zero padding relies
    on the runtime's pre-zeroed ExternalOutput buffers
"""

import sys

sys.path.insert(0, "/opt/trn_rl_repo")

import numpy as np

B, E, N, T, F, PAD = 8, 64, 512, 2048, 32768, 65536
L = 16384  # outputs per (event, half) partition
KW = 66  # sample window per 1024-output interp chunk
NCORES = 8

_cache = {}


def _build_winterp() -> np.ndarray:
    """Constant [66, 1024] triangle-filter matrix for 16x linear upsampling.

    Output local index q = 16*m + j consumes window samples s' = m + {0,1,2}:
      j < 8 : taps (15-2j)/32 on m,   (17+2j)/32 on m+1
      j >= 8: taps (47-2j)/32 on m+1, (2j-15)/32 on m+2
    """
    w = np.zeros((KW, 1024), np.float32)
    for q in range(1024):
        m, j = divmod(q, 16)
        if j < 8:
            w[m, q] = (15 - 2 * j) / 32.0
            w[m + 1, q] = (17 + 2 * j) / 32.0
        else:
            w[m + 1, q] = (47 - 2 * j) / 32.0
            w[m + 2, q] = (2 * j - 15) / 32.0
    return w


def _declare_io(nc):
    import concourse.mybir as mybir

    f32 = mybir.dt.float32
    return dict(
        sel_ap=nc.dram_tensor("selections", [E, N], f32, kind="ExternalInput").ap(),
        items_ap=nc.dram_tensor("items", [N, T], f32, kind="ExternalInput").ap(),
        islice_ap=nc.dram_tensor("items_slice", [E, T], f32, kind="ExternalInput").ap(),
        noise_ap=nc.dram_tensor("noise", [E, F], f32, kind="ExternalInput").ap(),
        w_ap=nc.dram_tensor("winterp", [KW, 1024], f32, kind="ExternalInput").ap(),
        out_ap=nc.dram_tensor("out", [E, PAD], f32, kind="ExternalOutput").ap(),
    )


def _emit(tc, nc, io):
    import concourse.bass as bass
    import concourse.masks as masks
    import concourse.mybir as mybir
    from concourse import bass_isa

    f32 = mybir.dt.float32
    X = mybir.AxisListType.X
    OP = mybir.AluOpType
    ACT = mybir.ActivationFunctionType

    sel_ap = io["sel_ap"]
    items_ap = io["items_ap"]
    islice_ap = io["islice_ap"]
    noise_ap = io["noise_ap"]
    w_ap = io["w_ap"]
    out_ap = io["out_ap"]

    from contextlib import ExitStack

    ctx = ExitStack()
    const = ctx.enter_context(tc.tile_pool(name="const", bufs=1))
    stats = ctx.enter_context(tc.tile_pool(name="stats", bufs=1))
    psA = ctx.enter_context(tc.tile_pool(name="psA", bufs=2, space="PSUM"))
    psI = ctx.enter_context(tc.tile_pool(name="psI", bufs=3, space="PSUM"))
    noisep = ctx.enter_context(tc.tile_pool(name="noisep", bufs=8))
    outp = ctx.enter_context(tc.tile_pool(name="outp", bufs=4))
    t1p = ctx.enter_context(tc.tile_pool(name="t1p", bufs=6))
    tmpp = ctx.enter_context(tc.tile_pool(name="tmpp", bufs=3))

    ident = const.tile([128, 128], f32, tag="ident")
    masks.make_identity(nc, ident[:])

    # Small loads in critical-chain order on the sync HWDGE FIFO:
    # sel64 gates argmax -> gather -> window tiles; islice gates the
    # min/max collective; seldup/W are needed later.
    sel64 = const.tile([E, N], f32, tag="sel64")
    nc.sync.dma_start(sel64[:], sel_ap[:])
    isl = const.tile([E, T], f32, tag="islice")
    nc.sync.dma_start(isl[:], islice_ap[:])
    seldup = const.tile([128, N], f32, tag="seldup")
    nc.sync.dma_start(seldup[:], sel_ap[:, None, :].broadcast_to([E, 2, N]))
    w_sb = const.tile([KW, 1024], f32, tag="winterp")
    nc.sync.dma_start(w_sb[:], w_ap[:])

    def softmax_stats(src, parts, tagp):
        m = stats.tile([parts, 1], f32, tag=f"m{tagp}")
        nc.vector.tensor_reduce(m[:], src[:], axis=X, op=OP.max)
        negm = stats.tile([parts, 1], f32, tag=f"negm{tagp}")
        nc.vector.tensor_scalar_mul(negm[:], m[:], -1.0)
        ex = const.tile([parts, N], f32, tag=f"exp{tagp}")
        s = stats.tile([parts, 1], f32, tag=f"s{tagp}")
        nc.scalar.activation(ex[:], src[:], ACT.Exp, bias=negm[:], scale=1.0, accum_out=s[:])
        w = stats.tile([parts, 1], f32, tag=f"w{tagp}")
        nc.vector.reciprocal(w[:], s[:])
        return m, w

    m64, w64 = softmax_stats(sel64, E, "64")
    _m128, w128 = softmax_stats(seldup, 128, "128")

    # ---- argmax row indices (top-8 then index-of) ----
    mx8 = stats.tile([E, 8], f32, tag="mx8")
    nc.vector.max(mx8[:], sel64[:])
    idx8 = stats.tile([E, 8], mybir.dt.uint32, tag="idx8")
    nc.vector.max_index(idx8[:], mx8[:], sel64[:])

    # ---- distributed items min/max: this core scans rows [64b, 64b+64),
    # then one tiny AllReduce(max) over [max, -min] across the 8 cores ----
    mn_l = stats.tile([E, 1], f32, tag="mn_l")
    mx_l = stats.tile([E, 1], f32, tag="mx_l")
    nc.vector.tensor_reduce(mn_l[:], isl[:], axis=X, op=OP.min)
    nc.vector.tensor_reduce(mx_l[:], isl[:], axis=X, op=OP.max)
    negmn_l = stats.tile([E, 1], f32, tag="negmn_l")
    nc.vector.tensor_scalar_mul(negmn_l[:], mn_l[:], -1.0)
    negmn_a = stats.tile([E, 1], f32, tag="negmn_a")
    nc.gpsimd.partition_all_reduce(negmn_a[:], negmn_l[:], channels=E,
                                   reduce_op=bass_isa.ReduceOp.max)
    mx_a = stats.tile([E, 1], f32, tag="mx_a")
    nc.gpsimd.partition_all_reduce(mx_a[:], mx_l[:], channels=E,
                                   reduce_op=bass_isa.ReduceOp.max)
    pk = stats.tile([1, 2], f32, tag="pk")
    nc.vector.tensor_copy(pk[0:1, 0:1], mx_a[0:1, 0:1])
    nc.vector.tensor_copy(pk[0:1, 1:2], negmn_a[0:1, 0:1])
    dramp = ctx.enter_context(tc.tile_pool(name="dramp", bufs=1, space="DRAM"))
    cin = dramp.tile([1, 2], f32, tag="cin")
    cout = dramp.tile([1, 2], f32, tag="cout")
    nc.gpsimd.dma_start(cin[:], pk[:])
    nc.gpsimd.collective_compute(
        "AllReduce", OP.max, replica_groups=[list(range(NCORES))],
        ins=[cin.opt()], outs=[cout.opt()])
    pk2 = stats.tile([1, 2], f32, tag="pk2")
    nc.gpsimd.dma_start(pk2[:], cout[:])
    pkb = stats.tile([128, 2], f32, tag="pkb")
    nc.gpsimd.partition_broadcast(pkb[:], pk2[:])
    mx = pkb[:, 0:1]
    negmn = pkb[:, 1:2]
    diffp = stats.tile([128, 1], f32, tag="diffp")
    nc.vector.tensor_scalar(diffp[:], mx[:], negmn[:], 0.001, op0=OP.add, op1=OP.add)
    inv = stats.tile([128, 1], f32, tag="inv")
    nc.vector.reciprocal(inv[:], diffp[:])
    negmninv = stats.tile([128, 1], f32, tag="negmninv")
    nc.vector.tensor_scalar(negmninv[:], negmn[:], inv[:], None, op0=OP.mult)
    bias128 = stats.tile([128, 1], f32, tag="bias128")
    nc.vector.tensor_scalar(bias128[:], w128[:], negmninv[:], None, op0=OP.mult)

    # ---- gather the argmax rows (one row per event partition) + halo pad,
    # scale by w[e] ----
    g_sb = const.tile([E, T + 2], f32, tag="gath")
    nc.gpsimd.indirect_dma_start(
        out=g_sb[:, 1:T + 1], out_offset=None, in_=items_ap[:],
        in_offset=bass.IndirectOffsetOnAxis(ap=idx8[:, 0:1], axis=0))
    nc.vector.tensor_copy(g_sb[:, 0:1], g_sb[:, 1:2])
    nc.vector.tensor_copy(g_sb[:, T + 1:T + 2], g_sb[:, T:T + 1])
    gsc = const.tile([E, T + 2], f32, tag="gsc")
    nc.vector.tensor_scalar(gsc[:], g_sb[:], w64[:], None, op0=OP.mult)

    # ---- window tiles via PE transpose:
    # T[ci][r, 2e+h] = w[e] * items_halo[row_e, 1024h + 64ci + r]
    T_sb = [const.tile([KW, 128], f32, tag=f"T{k}", name=f"T{k}") for k in range(16)]
    for ci in range(16):
        for h in range(2):
            ps = psA.tile([KW, E], f32, tag="ps_sel")
            c0 = 1024 * h + 64 * ci
            nc.tensor.transpose(ps[:], gsc[:, c0:c0 + KW], ident[:E, :E])
            nc.scalar.copy(T_sb[ci][0:KW, h:128:2], ps[:])

    # ---- main loop: interp matmuls + (psum + bias) * noise, stream out ----
    noise3 = noise_ap.rearrange("e (h x) -> e h x", h=2)
    outlive3 = out_ap[:, 0:F].rearrange("e (h x) -> e h x", h=2)
    for cd in range(8):
        nt = noisep.tile([128, 2048], f32, tag="nt")
        nc.sync.dma_start(nt[:], noise3[:, :, 2048 * cd:2048 * (cd + 1)])
        ot = outp.tile([128, 2048], f32, tag="ot")
        for sub in range(2):
            ci = 2 * cd + sub
            ps = psI.tile([128, 1024], f32, tag="ps_interp")
            for nn in range(2):
                sl = slice(nn * 512, (nn + 1) * 512)
                nc.tensor.matmul(ps[:, sl], T_sb[ci][0:KW, :],
                                 w_sb[0:KW, sl], start=True, stop=True)
            # psum drains without waiting on the min/max collective:
            #   out = (psum*noise)*inv + bias*noise
            nsub = nt[:, sub * 1024:(sub + 1) * 1024]
            t1 = t1p.tile([128, 1024], f32, tag="t1")
            nc.vector.tensor_mul(t1[:], ps[:], nsub)
            u = tmpp.tile([128, 1024], f32, tag="tmp")
            nc.scalar.activation(u[:], t1[:], ACT.Copy, scale=inv[:])
            nc.vector.scalar_tensor_tensor(
                ot[:, sub * 1024:(sub + 1) * 1024], nsub, bias128[:], u[:],
                op0=OP.mult, op1=OP.add)
        if cd < 7:
            nc.scalar.dma_start(outlive3[:, :, 2048 * cd:2048 * (cd + 1)], ot[:])
        else:
            # last chunk: write each 1024-half as soon as it is ready to
            # shorten the compute-chain tail
            for sub in range(2):
                nc.scalar.dma_start(
                    outlive3[:, :, 2048 * cd + 1024 * sub:2048 * cd + 1024 * (sub + 1)],
                    ot[:, 1024 * sub:1024 * (sub + 1)])

    # Zero padding (out[:, F:PAD]) is not written: both run_neff and the
    # PJRT donation path hand the kernel pre-zeroed ExternalOutput buffers.

    ctx.close()


def _program_bench(reps: int = 1, barrier: bool = False):
    """Timing-only variant: all real I/O lives in Internal DRAM (no host
    transfer), one dummy external in/out so the PJRT path has operands.
    barrier=True serializes reps (per-rep = single-shot latency)."""
    key = ("bench", reps, barrier)
    if key in _cache:
        return _cache[key]
    import concourse.bacc as bacc
    import concourse.mybir as mybir
    import concourse.tile as tile

    f32 = mybir.dt.float32
    nc = bacc.Bacc("TRN2", target_bir_lowering=False, debug=False,
                   num_devices=NCORES)
    dummy_in = nc.dram_tensor("bench_in", [1, 128], f32, kind="ExternalInput").ap()
    dummy_out = nc.dram_tensor("bench_out", [1, 128], f32, kind="ExternalOutput").ap()
    io = dict(
        sel_ap=nc.dram_tensor("selections", [E, N], f32).ap(),
        items_ap=nc.dram_tensor("items", [N, T], f32).ap(),
        islice_ap=nc.dram_tensor("items_slice", [E, T], f32).ap(),
        noise_ap=nc.dram_tensor("noise", [E, F], f32).ap(),
        w_ap=nc.dram_tensor("winterp", [KW, 1024], f32).ap(),
        out_ap=nc.dram_tensor("out", [E, PAD], f32).ap(),
    )
    with tile.TileContext(nc) as tc:
        for i in range(reps):
            if barrier and i:
                tc.strict_bb_all_engine_barrier()
            _emit(tc, nc, io)
        with tc.tile_pool(name="dummyp", bufs=1) as dp:
            t = dp.tile([1, 128], f32)
            nc.sync.dma_start(t[:], dummy_in[:])
            nc.sync.dma_start(dummy_out[:], t[:])
    nc.compile()
    _cache[key] = nc
    return nc


def _program(reps: int = 1):
    key = ("nc", reps)
    if key in _cache:
        return _cache[key]
    import concourse.bacc as bacc
    import concourse.tile as tile

    nc = bacc.Bacc("TRN2", target_bir_lowering=False, debug=False,
                   num_devices=NCORES)
    io = _declare_io(nc)
    with tile.TileContext(nc) as tc:
        for _ in range(reps):
            _emit(tc, nc, io)
    nc.compile()
    _cache[key] = nc
    return nc


def kernel(selections: np.ndarray, items: np.ndarray, noise: np.ndarray) -> np.ndarray:
    from concourse.bass_utils import run_bass_kernel_spmd

    nc = _program()
    winterp = _build_winterp()
    sel = np.ascontiguousarray(np.asarray(selections, np.float32))
    it = np.ascontiguousarray(np.asarray(items, np.float32))
    nz = np.ascontiguousarray(np.asarray(noise, np.float32))
    in_maps = [
        {"selections": sel[b], "items": it,
         "items_slice": it[E * b:E * (b + 1)],
         "noise": nz[b], "winterp": winterp}
        for b in range(NCORES)
    ]
    last_err = None
    for attempt in range(3):
        try:
            res = run_bass_kernel_spmd(nc, in_maps, list(range(NCORES)))
            break
        except Exception as e:  # transient NRT device wedge: retry
            last_err = e
            import time as _time

            _time.sleep(2.0 * (attempt + 1))
    else:
        raise last_err
    return np.stack([res.results[b]["out"] for b in range(NCORES)]).astype(np.float32)
